# revision 32
# baseline (speedup 1.0000x reference)
"""CP-decomposed 3x3 conv on 8 TRN2 NeuronCores — K-packed two-stage matmul.

Math: out[f,i,j] = sum_{h,w,c,r} in[c,i+h,j+w] * f1[h,r] * f2[w,r] * f3[c,r] * f0[f,r]

Per-core factorization (over its 32 output rows, n flattens (row,col), pitch
W=256; cols 254/255 of each row are garbage, dropped by output compaction):

  stage A: t2[r,n] = sum_h sum_c (f3[c,r]*f1[h,r]) * x[c, n+h*W]
    packed as ONE K=128 matmul (h=0,1 stacked: partition half 1 of the input
    holds x shifted by one row, built on host) + ONE K=64 matmul (h=2 at col
    offset 2W on half 0).
  stage B: out[f,n] = sum_w sum_r (f2[w,r]*f0[f,r]) * t2[r, n+w]
    packed as ONE K=128 matmul (w=0,1 stacked: t2 half 1 holds t2 shifted by
    one col, produced by a fast SBUF->SBUF bf16 copy) + ONE K=64 matmul
    (w=2 reads the shifted half at +1).

4 matmuls per 512-col chunk, 64 per core: ~matmul-roofline for this CP shape.
Inputs are bf16 (host-converted); output is stored compacted (254 cols) as
bf16 and widened on host.

Sharding: output rows (Ho=254) split across 8 cores: cores 0-6 get rows
[32i, 32i+32); core 7 processes rows [222, 254) via a shifted window (its
first 2 rows duplicate core 6's tail and are dropped at gather).
"""

import sys

sys.path.insert(0, "/opt/trn_rl_repo")

import numpy as np

# Problem constants (hardcoded per contract)
C = 64
H = 256
W = 256
FH = 3
FW = 3
RANK = 64
F = 128
HO = H - FH + 1  # 254
WO = W - FW + 1  # 254
NCORES = 8
ROWS = 32  # output rows per core
IN_ROWS = ROWS + 2
NCOLS = IN_ROWS * W  # 8704
CHUNK = 512  # one PSUM bank of fp32: 2 output rows x 256
PAIR = 2 * CHUNK  # chunk pair processed together
NPAIR = ROWS * W // PAIR  # 8

_PROGRAM_CACHE = {}


def _bf16():
    import ml_dtypes

    return np.dtype(ml_dtypes.bfloat16)


def build_program(rows=ROWS, num_devices=NCORES, reps=1):
    """Build + compile the per-core Bass program."""
    from concourse import bacc, mybir, tile

    dt_c = mybir.dt.bfloat16
    dt_f32 = mybir.dt.float32

    in_rows = rows + 2
    ncols = in_rows * W  # 8704
    npair = rows * W // PAIR

    nc = bacc.Bacc(
        "TRN2", target_bir_lowering=False, debug=False, num_devices=num_devices
    )
    # x2: half0 = x rows 0..in_rows, half1 = x shifted one row (host-built).
    x = nc.dram_tensor("x", [2 * C, ncols], dt_c, kind="ExternalInput").ap()
    w = nc.dram_tensor("w", [F, 384], dt_c, kind="ExternalInput").ap()
    y = nc.dram_tensor("y", [F, rows, WO], dt_c, kind="ExternalOutput").ap()

    with tile.TileContext(nc) as tc:
        with (
            tc.tile_pool(name="xin", bufs=1) as xin_pool,
            tc.tile_pool(name="wgt", bufs=1) as wgt_pool,
            tc.tile_pool(name="t2", bufs=5) as t2_pool,
            tc.tile_pool(name="ot", bufs=4) as ot_pool,
            tc.tile_pool(name="p1", bufs=2, space="PSUM") as p1_pool,
            tc.tile_pool(name="p2a", bufs=2, space="PSUM") as p2a_pool,
            tc.tile_pool(name="p2b", bufs=2, space="PSUM") as p2b_pool,
        ):

            def body():
                X = xin_pool.tile([2 * C, ncols], dt_c)
                WT = wgt_pool.tile([F, 384], dt_c)
                # Weights first on SP (small); Pool starts slab0 in parallel.
                nc.sync.dma_start(out=WT[:], in_=w[:])
                WA1 = WT[:, 0:64]  # K=128 (h=0,1), M=64
                WA2 = WT[0:64, 64:128]  # K=64  (h=2),   M=64
                WB1 = WT[:, 128:256]  # K=128 (w=0,1), M=128
                WB2 = WT[64:128, 256:384]  # K=64 (w=2), M=128 (parts 64+)

                # Input slabs: full-partition-width col slices (fast DMA
                # path) on THREE queues (Act's queue is free until its first
                # compute op), paced to PE's A-stage demand: A(p) needs cols
                # < 1536 + 1024p.
                for eng, a, b in (
                    (nc.gpsimd, 0, 6 * W),  # ready ~2.2us (A0, A1)
                    (nc.sync, 6 * W, 14 * W),  # ready ~3.2us (A2, A3)
                    (nc.gpsimd, 14 * W, 22 * W),  # ready ~3.8us (A4, A5)
                    (nc.sync, 22 * W, 28 * W),  # ready ~4.4us (A6)
                    (nc.gpsimd, 28 * W, ncols),  # ready ~5.0us (A7)
                ):
                    eng.dma_start(out=X[:, a:b], in_=x[:, a:b])

                p1_tiles = {}

                def stage_a(p):
                    b0 = p * PAIR
                    # Two pairs share one [128, PAIR] PSUM tile (partition
                    # halves) so 4 pairs fit in 4 banks with bufs=2.
                    if p % 2 == 0:
                        p1_tiles[p // 2] = p1_pool.tile(
                            [2 * C, PAIR], dt_f32, tag="p1", name=f"p1_{p // 2}"
                        )
                    r0 = (p % 2) * C
                    p1 = p1_tiles[p // 2][r0 : r0 + C, :]
                    for g in range(2):
                        gc = g * CHUNK
                        nc.tensor.matmul(
                            out=p1[:, gc : gc + CHUNK],
                            lhsT=WA1,
                            rhs=X[:, b0 + gc : b0 + gc + CHUNK],
                            start=True,
                            stop=False,
                            skip_group_check=True,
                        )
                        nc.tensor.matmul(
                            out=p1[:, gc : gc + CHUNK],
                            lhsT=WA2,
                            rhs=X[0:C, b0 + gc + 2 * W : b0 + gc + 2 * W + CHUNK],
                            start=False,
                            stop=True,
                            skip_group_check=True,
                        )
                    # t2 tile: half0 = t2[b0 .. b0+1024] (+2 pad), half1 =
                    # t2 shifted by one col.
                    t2 = t2_pool.tile([2 * RANK, PAIR + 2], dt_c, tag="t2")
                    nc.vector.memset(t2[0:RANK, PAIR : PAIR + 2], 0.0)
                    if p < 2:
                        # Head of pipeline: split copy1a across Act+DVE so
                        # the p1 tile recycles before A(p+4) needs it.
                        nc.scalar.copy(
                            out=t2[0:RANK, 0:CHUNK], in_=p1[:, 0:CHUNK]
                        )
                        nc.vector.tensor_copy(
                            out=t2[0:RANK, CHUNK:PAIR], in_=p1[:, CHUNK:PAIR]
                        )
                    else:
                        nc.scalar.copy(out=t2[0:RANK, 0:PAIR], in_=p1[:])
                    nc.vector.tensor_copy(
                        out=t2[RANK : 2 * RANK, 0 : PAIR + 1],
                        in_=t2[0:RANK, 1 : PAIR + 2],
                    )
                    return t2

                def stage_b(p, t2):
                    # One PSUM bank tile per chunk so each bank evacuates
                    # (and recycles) independently right after its 2 matmuls.
                    p2 = []
                    for g in range(2):
                        gc = g * CHUNK
                        pool = p2a_pool if g == 0 else p2b_pool
                        pt = pool.tile(
                            [F, CHUNK], dt_f32, tag=f"p2{g}", name=f"p2_{p}_{g}"
                        )
                        nc.tensor.matmul(
                            out=pt[:],
                            lhsT=WB1,
                            rhs=t2[:, gc : gc + CHUNK],
                            start=True,
                            stop=False,
                            skip_group_check=True,
                        )
                        nc.tensor.matmul(
                            out=pt[:],
                            lhsT=WB2,
                            rhs=t2[RANK : 2 * RANK, gc + 1 : gc + 1 + CHUNK],
                            start=False,
                            stop=True,
                            skip_group_check=True,
                        )
                        p2.append(pt)
                    return p2

                def evac(p, p2, ot, orow):
                    # Compact 256->254 cols while evacuating PSUM. Each bank
                    # half on its own engine, gated only on its own 2 matmuls,
                    # so banks recycle fast and stage B never waits.
                    dst = ot.rearrange("f (r w) -> f r w", w=WO)
                    for g in range(2):
                        src = p2[g].rearrange("f (r w) -> f r w", w=W)[:, :, 0:WO]
                        nc.vector.tensor_copy(
                            out=dst[:, orow + 2 * g : orow + 2 * g + 2, :], in_=src
                        )

                # Output staging: pairs 0-2 and 3-5 accumulate in two 12-row
                # tiles stored with one DMA each; the last two pairs store
                # per-pair (on the otherwise-idle Act queue) for a short tail.
                otbig = {}
                for hf in range(2):
                    otbig[hf] = ot_pool.tile(
                        [F, 12 * WO], dt_c, tag=f"otbig{hf}", name=f"otbig{hf}"
                    )
                yflat = y.rearrange("f r w -> f (r w)")

                # Software pipeline: stage A runs L pairs ahead of stage B so
                # the copy1a->copy1b chain never stalls PE.
                L = 4
                t2s = {}
                for p in range(npair + L):
                    if p < npair:
                        t2s[p] = stage_a(p)
                    if p >= L:
                        pp = p - L
                        p2 = stage_b(pp, t2s.pop(pp))
                        if pp < 6:
                            evac(pp, p2, otbig[pp // 3], 4 * (pp % 3))
                            if pp % 3 == 2:
                                eng = nc.sync
                                half = pp // 3
                                eng.dma_start(
                                    out=yflat[
                                        :, half * 12 * WO : (half + 1) * 12 * WO
                                    ],
                                    in_=otbig[half][:],
                                )
                        else:
                            ot = ot_pool.tile(
                                [F, 4 * WO], dt_c, tag=f"ot{pp}", name=f"ot_{pp}"
                            )
                            evac(pp, p2, ot, 0)
                            c0 = pp * 4 * WO
                            if pp < npair - 1:
                                nc.gpsimd.dma_start(
                                    out=yflat[:, c0 : c0 + 4 * WO], in_=ot[:]
                                )
                            else:
                                # Last pair: two 2-row stores on Pool (free at
                                # the tail; SP still drains the big store).
                                nc.gpsimd.dma_start(
                                    out=yflat[:, c0 : c0 + 2 * WO],
                                    in_=ot[:, 0 : 2 * WO],
                                )
                                nc.gpsimd.dma_start(
                                    out=yflat[:, c0 + 2 * WO : c0 + 4 * WO],
                                    in_=ot[:, 2 * WO : 4 * WO],
                                )

            if reps == 1:
                body()
            else:
                with tc.For_i(0, reps, 1):
                    body()

    nc.compile()
    return nc


def _get_program():
    key = (ROWS, NCORES)
    if key not in _PROGRAM_CACHE:
        _PROGRAM_CACHE[key] = build_program()
    return _PROGRAM_CACHE[key]


def make_weight_inputs(factor0, factor1, factor2, factor3):
    """Pack all four lhsT weight blocks into one [128, 384] bf16 tensor."""
    bf16 = _bf16()
    f0 = np.asarray(factor0, np.float32)
    f1 = np.asarray(factor1, np.float32)
    f2 = np.asarray(factor2, np.float32)
    f3 = np.asarray(factor3, np.float32)
    wall = np.zeros((F, 384), np.float32)
    # waK[h*64+c, r] = f1[h,r]*f3[c,r] for h in {0,1}
    wall[:, 0:64] = (f1[0:2, None, :] * f3[None, :, :]).reshape(F, RANK)
    # wa2[c, r] = f1[2,r]*f3[c,r]
    wall[0:C, 64:128] = f1[2][None, :] * f3
    # wbK[w*64+r, f] = f2[w,r]*f0[f,r] for w in {0,1}
    wall[:, 128:256] = (f2[0:2, :, None] * f0.T[None, :, :]).reshape(F, F)
    # wb2[r, f] = f2[2,r]*f0[f,r] — in partitions 64:128 to match its rhs
    # (t2 shifted half), since PE requires lhsT/rhs base partitions to match.
    wall[C:F, 256:384] = f2[2][:, None] * f0.T
    return np.ascontiguousarray(wall.astype(bf16))


ROW_STARTS = [0, 32, 64, 96, 128, 160, 192, 222]


def make_in_maps(input, factor0, factor1, factor2, factor3):
    bf16 = _bf16()
    wall = make_weight_inputs(factor0, factor1, factor2, factor3)
    x16 = np.asarray(input, np.float32).astype(bf16)
    maps = []
    for s in ROW_STARTS:
        x2 = np.zeros((2 * C, NCOLS), bf16)
        x2[0:C, :] = x16[:, s : s + IN_ROWS, :].reshape(C, NCOLS)
        x2[C : 2 * C, 0 : NCOLS - W] = x16[:, s + 1 : s + IN_ROWS, :].reshape(
            C, NCOLS - W
        )
        maps.append({"x": x2, "w": wall})
    return maps


def kernel(input, factor0, factor1, factor2, factor3):
    from concourse.bass_utils import run_bass_kernel_spmd

    nc = _get_program()
    in_maps = make_in_maps(input, factor0, factor1, factor2, factor3)
    res = run_bass_kernel_spmd(nc, in_maps, list(range(NCORES))).results
    out = np.empty((F, HO, WO), np.float32)
    for i, s in enumerate(ROW_STARTS):
        ys = np.asarray(res[i]["y"], np.float32).reshape(F, ROWS, WO)
        if i < NCORES - 1:
            out[:, s : s + ROWS, :] = ys
        else:
            out[:, 224:HO, :] = ys[:, 2:ROWS, :]
    return out


# revision 36
# speedup vs baseline: 1.2376x; 1.2376x over previous
"""CP-decomposed 3x3 conv on 8 TRN2 NeuronCores — K-packed two-stage matmul.

Math: out[f,i,j] = sum_{h,w,c,r} in[c,i+h,j+w] * f1[h,r] * f2[w,r] * f3[c,r] * f0[f,r]

Per-core factorization (over its 32 output rows, n flattens (row,col), pitch
W=256; cols 254/255 of each row are garbage, dropped by output compaction):

  stage A: t2[r,n] = sum_h sum_c (f3[c,r]*f1[h,r]) * x[c, n+h*W]
    packed as ONE K=128 matmul (h=0,1 stacked: partition half 1 of the input
    holds x shifted by one row, built on host) + ONE K=64 matmul (h=2 at col
    offset 2W on half 0).
  stage B: out[f,n] = sum_w sum_r (f2[w,r]*f0[f,r]) * t2[r, n+w]
    packed as ONE K=128 matmul (w=0,1 stacked: t2 half 1 holds t2 shifted by
    one col, produced by a fast SBUF->SBUF bf16 copy) + ONE K=64 matmul
    (w=2 reads the shifted half at +1).

4 matmuls per 512-col chunk, 64 per core: ~matmul-roofline for this CP shape.
Inputs are bf16 (host-converted); output is stored compacted (254 cols) as
bf16 and widened on host.

Sharding: output rows (Ho=254) split across 8 cores: cores 0-6 get rows
[32i, 32i+32); core 7 processes rows [222, 254) via a shifted window (its
first 2 rows duplicate core 6's tail and are dropped at gather).
"""

import sys

sys.path.insert(0, "/opt/trn_rl_repo")

import numpy as np

# Problem constants (hardcoded per contract)
C = 64
H = 256
W = 256
FH = 3
FW = 3
RANK = 64
F = 128
HO = H - FH + 1  # 254
WO = W - FW + 1  # 254
NCORES = 8
ROWS = 32  # output rows per core
IN_ROWS = ROWS + 2
NCOLS = IN_ROWS * W  # 8704
CHUNK = 512  # one PSUM bank of fp32: 2 output rows x 256
PAIR = 2 * CHUNK  # chunk pair processed together
NPAIR = ROWS * W // PAIR  # 8

_PROGRAM_CACHE = {}


def _bf16():
    import ml_dtypes

    return np.dtype(ml_dtypes.bfloat16)


def build_program(rows=ROWS, num_devices=NCORES, reps=1):
    """Build + compile the per-core Bass program."""
    from concourse import bacc, mybir, tile

    dt_c = mybir.dt.bfloat16
    dt_f32 = mybir.dt.float32

    in_rows = rows + 2
    ncols = in_rows * W  # 8704
    npair = rows * W // PAIR

    nc = bacc.Bacc(
        "TRN2", target_bir_lowering=False, debug=False, num_devices=num_devices
    )
    # x2: half0 = x rows 0..in_rows, half1 = x shifted one row (host-built).
    x = nc.dram_tensor("x", [2 * C, ncols], dt_c, kind="ExternalInput").ap()
    w = nc.dram_tensor("w", [F, 384], dt_c, kind="ExternalInput").ap()
    y = nc.dram_tensor("y", [F, rows, WO], dt_c, kind="ExternalOutput").ap()

    with tile.TileContext(nc) as tc:
        with (
            tc.tile_pool(name="xin", bufs=1) as xin_pool,
            tc.tile_pool(name="wgt", bufs=1) as wgt_pool,
            tc.tile_pool(name="t2", bufs=5) as t2_pool,
            tc.tile_pool(name="ot", bufs=4) as ot_pool,
            tc.tile_pool(name="p1", bufs=2, space="PSUM") as p1_pool,
            tc.tile_pool(name="p2a", bufs=2, space="PSUM") as p2a_pool,
            tc.tile_pool(name="p2b", bufs=2, space="PSUM") as p2b_pool,
        ):

            def body():
                X = xin_pool.tile([2 * C, ncols], dt_c)
                WT = wgt_pool.tile([F, 384], dt_c)
                # Weights first on SP (small); Pool starts slab0 in parallel.
                nc.sync.dma_start(out=WT[:], in_=w[:])
                WA1 = WT[:, 0:64]  # K=128 (h=0,1), M=64
                WA2 = WT[0:64, 64:128]  # K=64  (h=2),   M=64
                WB1 = WT[:, 128:256]  # K=128 (w=0,1), M=128
                WB2 = WT[64:128, 256:384]  # K=64 (w=2), M=128 (parts 64+)

                # Input slabs: full-partition-width col slices (fast DMA
                # path) on THREE queues (Act's queue is free until its first
                # compute op), paced to PE's A-stage demand: A(p) needs cols
                # < 1536 + 1024p.
                for eng, a, b in (
                    (nc.gpsimd, 0, 6 * W),  # ready ~2.2us (A0, A1)
                    (nc.scalar, 6 * W, 14 * W),  # ready ~2.6us (A2, A3)
                    (nc.sync, 14 * W, 22 * W),  # ready ~3.2us (A4, A5)
                    (nc.sync, 22 * W, 28 * W),  # ready ~4.4us (A6)
                    (nc.gpsimd, 28 * W, ncols),  # ready ~3.4us (A7)
                ):
                    eng.dma_start(out=X[:, a:b], in_=x[:, a:b])

                p1_tiles = {}

                def stage_a(p):
                    b0 = p * PAIR
                    # Two pairs share one [128, PAIR] PSUM tile (partition
                    # halves) so 4 pairs fit in 4 banks with bufs=2.
                    if p % 2 == 0:
                        p1_tiles[p // 2] = p1_pool.tile(
                            [2 * C, PAIR], dt_f32, tag="p1", name=f"p1_{p // 2}"
                        )
                    r0 = (p % 2) * C
                    p1 = p1_tiles[p // 2][r0 : r0 + C, :]
                    for g in range(2):
                        gc = g * CHUNK
                        nc.tensor.matmul(
                            out=p1[:, gc : gc + CHUNK],
                            lhsT=WA1,
                            rhs=X[:, b0 + gc : b0 + gc + CHUNK],
                            start=True,
                            stop=False,
                            skip_group_check=True,
                        )
                        nc.tensor.matmul(
                            out=p1[:, gc : gc + CHUNK],
                            lhsT=WA2,
                            rhs=X[0:C, b0 + gc + 2 * W : b0 + gc + 2 * W + CHUNK],
                            start=False,
                            stop=True,
                            skip_group_check=True,
                        )
                    # t2 tile: half0 = t2[b0 .. b0+1024] (+2 pad), half1 =
                    # t2 shifted by one col.
                    t2 = t2_pool.tile([2 * RANK, PAIR + 2], dt_c, tag="t2")
                    nc.gpsimd.memset(t2[0:RANK, PAIR : PAIR + 2], 0.0)
                    if p < 2:
                        # Head of pipeline: split copy1a across Act+DVE so
                        # the p1 tile recycles before A(p+4) needs it.
                        nc.scalar.copy(
                            out=t2[0:RANK, 0:CHUNK], in_=p1[:, 0:CHUNK]
                        )
                        nc.vector.tensor_copy(
                            out=t2[0:RANK, CHUNK:PAIR], in_=p1[:, CHUNK:PAIR]
                        )
                    else:
                        nc.scalar.copy(out=t2[0:RANK, 0:PAIR], in_=p1[:])
                    nc.vector.tensor_copy(
                        out=t2[RANK : 2 * RANK, 0 : PAIR + 1],
                        in_=t2[0:RANK, 1 : PAIR + 2],
                    )
                    return t2

                def stage_b(p, t2):
                    # One PSUM bank tile per chunk so each bank evacuates
                    # (and recycles) independently right after its 2 matmuls.
                    p2 = []
                    for g in range(2):
                        gc = g * CHUNK
                        pool = p2a_pool if g == 0 else p2b_pool
                        pt = pool.tile(
                            [F, CHUNK], dt_f32, tag=f"p2{g}", name=f"p2_{p}_{g}"
                        )
                        nc.tensor.matmul(
                            out=pt[:],
                            lhsT=WB1,
                            rhs=t2[:, gc : gc + CHUNK],
                            start=True,
                            stop=False,
                            skip_group_check=True,
                        )
                        nc.tensor.matmul(
                            out=pt[:],
                            lhsT=WB2,
                            rhs=t2[RANK : 2 * RANK, gc + 1 : gc + 1 + CHUNK],
                            start=False,
                            stop=True,
                            skip_group_check=True,
                        )
                        p2.append(pt)
                    return p2

                def evac(p, p2, ot, orow):
                    # Compact 256->254 cols while evacuating PSUM. Each bank
                    # half on its own engine, gated only on its own 2 matmuls,
                    # so banks recycle fast and stage B never waits.
                    dst = ot.rearrange("f (r w) -> f r w", w=WO)
                    for g in range(2):
                        src = p2[g].rearrange("f (r w) -> f r w", w=W)[:, :, 0:WO]
                        d = dst[:, orow + 2 * g : orow + 2 * g + 2, :]
                        if g == 0:
                            nc.scalar.copy(out=d, in_=src)
                        else:
                            nc.vector.tensor_copy(out=d, in_=src)

                # Output staging: pairs 0-2 and 3-5 accumulate in two 12-row
                # tiles stored with one DMA each; the last two pairs store
                # per-pair (on the otherwise-idle Act queue) for a short tail.
                otbig = {}
                for hf in range(2):
                    otbig[hf] = ot_pool.tile(
                        [F, 12 * WO], dt_c, tag=f"otbig{hf}", name=f"otbig{hf}"
                    )
                yflat = y.rearrange("f r w -> f (r w)")

                # Software pipeline: stage A runs L pairs ahead of stage B so
                # the copy1a->copy1b chain never stalls PE.
                L = 4
                t2s = {}
                for p in range(npair + L):
                    if p < npair:
                        t2s[p] = stage_a(p)
                    if p >= L:
                        pp = p - L
                        p2 = stage_b(pp, t2s.pop(pp))
                        if pp < 6:
                            evac(pp, p2, otbig[pp // 3], 4 * (pp % 3))
                            if pp % 3 == 2:
                                eng = nc.sync
                                half = pp // 3
                                eng.dma_start(
                                    out=yflat[
                                        :, half * 12 * WO : (half + 1) * 12 * WO
                                    ],
                                    in_=otbig[half][:],
                                )
                        else:
                            ot = ot_pool.tile(
                                [F, 4 * WO], dt_c, tag=f"ot{pp}", name=f"ot_{pp}"
                            )
                            evac(pp, p2, ot, 0)
                            c0 = pp * 4 * WO
                            if pp < npair - 1:
                                nc.gpsimd.dma_start(
                                    out=yflat[:, c0 : c0 + 4 * WO], in_=ot[:]
                                )
                            else:
                                # Last pair: two parallel 2-row stores on the
                                # queues free at the tail (SP still drains
                                # the second big store).
                                nc.gpsimd.dma_start(
                                    out=yflat[:, c0 : c0 + 2 * WO],
                                    in_=ot[:, 0 : 2 * WO],
                                )
                                nc.scalar.dma_start(
                                    out=yflat[:, c0 + 2 * WO : c0 + 4 * WO],
                                    in_=ot[:, 2 * WO : 4 * WO],
                                )

            if reps == 1:
                body()
            else:
                with tc.For_i(0, reps, 1):
                    body()

    nc.compile()
    return nc


def _get_program():
    key = (ROWS, NCORES)
    if key not in _PROGRAM_CACHE:
        _PROGRAM_CACHE[key] = build_program()
    return _PROGRAM_CACHE[key]


def make_weight_inputs(factor0, factor1, factor2, factor3):
    """Pack all four lhsT weight blocks into one [128, 384] bf16 tensor."""
    bf16 = _bf16()
    f0 = np.asarray(factor0, np.float32)
    f1 = np.asarray(factor1, np.float32)
    f2 = np.asarray(factor2, np.float32)
    f3 = np.asarray(factor3, np.float32)
    wall = np.zeros((F, 384), np.float32)
    # waK[h*64+c, r] = f1[h,r]*f3[c,r] for h in {0,1}
    wall[:, 0:64] = (f1[0:2, None, :] * f3[None, :, :]).reshape(F, RANK)
    # wa2[c, r] = f1[2,r]*f3[c,r]
    wall[0:C, 64:128] = f1[2][None, :] * f3
    # wbK[w*64+r, f] = f2[w,r]*f0[f,r] for w in {0,1}
    wall[:, 128:256] = (f2[0:2, :, None] * f0.T[None, :, :]).reshape(F, F)
    # wb2[r, f] = f2[2,r]*f0[f,r] — in partitions 64:128 to match its rhs
    # (t2 shifted half), since PE requires lhsT/rhs base partitions to match.
    wall[C:F, 256:384] = f2[2][:, None] * f0.T
    return np.ascontiguousarray(wall.astype(bf16))


ROW_STARTS = [0, 32, 64, 96, 128, 160, 192, 222]


def make_in_maps(input, factor0, factor1, factor2, factor3):
    bf16 = _bf16()
    wall = make_weight_inputs(factor0, factor1, factor2, factor3)
    x16 = np.asarray(input, np.float32).astype(bf16)
    maps = []
    for s in ROW_STARTS:
        x2 = np.zeros((2 * C, NCOLS), bf16)
        x2[0:C, :] = x16[:, s : s + IN_ROWS, :].reshape(C, NCOLS)
        x2[C : 2 * C, 0 : NCOLS - W] = x16[:, s + 1 : s + IN_ROWS, :].reshape(
            C, NCOLS - W
        )
        maps.append({"x": x2, "w": wall})
    return maps


def kernel(input, factor0, factor1, factor2, factor3):
    from concourse.bass_utils import run_bass_kernel_spmd

    nc = _get_program()
    in_maps = make_in_maps(input, factor0, factor1, factor2, factor3)
    res = run_bass_kernel_spmd(nc, in_maps, list(range(NCORES))).results
    out = np.empty((F, HO, WO), np.float32)
    for i, s in enumerate(ROW_STARTS):
        ys = np.asarray(res[i]["y"], np.float32).reshape(F, ROWS, WO)
        if i < NCORES - 1:
            out[:, s : s + ROWS, :] = ys
        else:
            out[:, 224:HO, :] = ys[:, 2:ROWS, :]
    return out
